# revision 5
# baseline (speedup 1.0000x reference)
"""Multi-head causal attention (B=2, T=2048, C=1024, H=16, HS=64) on 8 TRN2
NeuronCores.

Sharding: 2 heads per core (tensor parallel). Each core receives the full
(pre-transposed) activations xT [B, C, T], its 2 heads' QKV weight slices
packed [C, 128], and its 128-column slice of w_proj transposed [128, C].
Each core computes a partial output [B, T, C] in bf16; the host sums the 8
partials and adds b_proj.

Per-core kernel (matmuls in float32r):
  - QT/KT [128(2 heads x 64), T] via lhsT=weight chunks, rhs=xT chunks.
  - V_aug [keys, 2, j, 64]: V (vo=0, via PE-transpose of VT) | ones (vo=1).
  - Flash-style causal attention in transposed layout: S^T[keys, q] blocks
    via lhsT=KT block, rhs=QT slice; exp on ScalarE (no max subtraction --
    scores are O(1) by construction); O^T = [V|1].T @ P^T accumulated over
    key blocks gives both O rows (0:64) and the softmax sums l (64:128).
  - Triangular masking of diagonal blocks via bf16 multiply on GpSimd.
  - Normalize with reciprocal_approx_fast; proj via lhsT=OhatT chunks.

Scheduling: the PE p-state ramps to 2.4 GHz only after ~3us of gap-free
execution and resets on any idle, so the emission interleaves one "filler"
PE unit (a QKV 512-col chain or a proj chunk) into every attention jg slot,
and skews O^T one jg behind S^T/exp so the PE never waits on ScalarE.
"""

import math
import sys
from collections import deque
from contextlib import ExitStack

if "/opt/trn_rl_repo" not in sys.path:
    sys.path.insert(0, "/opt/trn_rl_repo")

import numpy as np

import concourse.mybir as mybir
import concourse.tile as tile
from concourse import bacc
from concourse.bass import ts
from concourse.bass_utils import run_bass_kernel_spmd
from concourse.tile_rust import add_dep_helper

B, T, C = 2, 2048, 1024
H, HS = 16, 64
NCORES = 8
HPC = H // NCORES  # heads per core
P = 128
G = 512  # q-group size
NG = T // G
KB = 128  # key block
NKB = T // KB
NPO = C // P  # contraction chunks
F32 = mybir.dt.float32
F32R = mybir.dt.float32r
BF16 = mybir.dt.bfloat16
SCALE = float(HS) ** -0.5

_nc_cache = {}


def _emit(tc):
    nc = tc.nc
    xt = nc.dram_tensor("xt", [B, C, T], F32R, kind="ExternalInput").ap()
    wq2 = nc.dram_tensor("wq2", [C, 128], F32R, kind="ExternalInput").ap()
    wk2 = nc.dram_tensor("wk2", [C, 128], F32R, kind="ExternalInput").ap()
    wv2 = nc.dram_tensor("wv2", [C, 128], F32R, kind="ExternalInput").ap()
    wpt = nc.dram_tensor("wpt", [128, C], F32R, kind="ExternalInput").ap()
    tri = nc.dram_tensor("tri", [P, P], BF16, kind="ExternalInput").ap()
    identd = nc.dram_tensor("ident", [P, 64], BF16, kind="ExternalInput").ap()
    onesd = nc.dram_tensor("ones", [P, NKB, 64], BF16, kind="ExternalInput").ap()
    out = nc.dram_tensor("out", [B, T, C], BF16, kind="ExternalOutput").ap()

    ctx = ExitStack()
    persist = ctx.enter_context(tc.tile_pool(name="persist", bufs=1))
    xt_pool = ctx.enter_context(tc.tile_pool(name="xtp", bufs=3))
    qk_pool = ctx.enter_context(tc.tile_pool(name="qkp", bufs=2))
    vt_pool = ctx.enter_context(tc.tile_pool(name="vtp", bufs=2))
    vaug_pool = ctx.enter_context(tc.tile_pool(name="vaugp", bufs=2))
    pt_pool = ctx.enter_context(tc.tile_pool(name="ptp", bufs=4))
    norm_pool = ctx.enter_context(tc.tile_pool(name="normp", bufs=2))
    ohat_pool = ctx.enter_context(tc.tile_pool(name="ohatp", bufs=2))
    out_pool = ctx.enter_context(tc.tile_pool(name="outp", bufs=3))
    st_psum = ctx.enter_context(tc.tile_pool(name="stps", bufs=2, space="PSUM"))
    ot_psum = ctx.enter_context(tc.tile_pool(name="otps", bufs=2, space="PSUM"))
    mm_psum = ctx.enter_context(tc.tile_pool(name="mmps", bufs=2, space="PSUM"))

    wq_sb = persist.tile([P, NPO, 128], F32R, tag="wq")
    wk_sb = persist.tile([P, NPO, 128], F32R, tag="wk")
    wv_sb = persist.tile([P, NPO, 128], F32R, tag="wv")
    wpt_sb = persist.tile([P, C], F32R, tag="wpt")
    tri_sb = persist.tile([P, P], BF16, tag="tri")
    ident = persist.tile([P, 64], BF16, tag="ident")

    nc.sync.dma_start(wq_sb[:], wq2.rearrange("(po pi) d -> pi po d", pi=P))
    nc.sync.dma_start(wk_sb[:], wk2.rearrange("(po pi) d -> pi po d", pi=P))
    nc.sync.dma_start(wv_sb[:], wv2.rearrange("(po pi) d -> pi po d", pi=P))
    nc.sync.dma_start(tri_sb[:], tri[:])
    nc.sync.dma_start(ident[:], identd[:])
    nc.sync.dma_start(wpt_sb[:], wpt[:])

    # ---- xt loading: per-tg tiles, loaded in two half-chunks (4 po each) ----
    xt_tiles = {}
    xt_dmas = []

    def load_xt_tg(b, tg):
        t = xt_pool.tile([P, NPO, 512], F32R, tag="xt", name=f"xt{b}{tg}")
        xt_tiles[(b, tg)] = t
        src = xt[b, :, ts(tg, 512)].rearrange("(po pi) t -> pi po t", pi=P)
        for half in range(2):
            po0 = 4 * half
            i = nc.sync.dma_start(t[:, po0 : po0 + 4, :], src[:, po0 : po0 + 4, :])
            if len(xt_dmas) >= 3:
                add_dep_helper(i.ins, xt_dmas[-3].ins, sync=True)
            xt_dmas.append(i)

    # just-in-time order: b0 tg0-2, then b1 tg0 before b0 tg3 so fillers
    # popped early in b0's attention never stall the PE queue
    for b, tg in [(0, 0), (0, 1), (0, 2), (1, 0), (0, 3), (1, 1), (1, 2), (1, 3)]:
        load_xt_tg(b, tg)

    def new_state(b):
        st = {
            "b": b,
            "qt": qk_pool.tile([P, T], F32R, tag="qt", name=f"qt{b}"),
            "kt": qk_pool.tile([P, T], F32R, tag="kt", name=f"kt{b}"),
            "vt": vt_pool.tile([P, T], BF16, tag="vt", name=f"vt{b}"),
            "ohat": ohat_pool.tile([P, T], F32R, tag="ohat", name=f"oh{b}"),
            "vaug": [],
        }
        for h in range(HPC):
            va = vaug_pool.tile(
                [P, NKB, 128], BF16, tag=f"vaug{h}", name=f"va{b}{h}"
            )
            i = nc.sync.dma_start(va[:, :, 64:128], onesd[:])
            add_dep_helper(i.ins, xt_dmas[0].ins, sync=True)
            st["vaug"].append(va)
        return st

    # ---------- building blocks ----------
    def emit_qkv_group(st, which, tg, copy_eng):
        w_sb, dst = {
            "q": (wq_sb, st["qt"]),
            "k": (wk_sb, st["kt"]),
            "v": (wv_sb, st["vt"]),
        }[which]
        ps = mm_psum.tile([P, 512], F32, tag="mm", name=f"qkv{which}{tg}")
        xtt = xt_tiles[(st["b"], tg)]
        for po in range(NPO):
            nc.tensor.matmul(
                ps[:],
                w_sb[:, po, :],
                xtt[:, po, :],
                start=(po == 0),
                stop=(po == NPO - 1),
            )
        if copy_eng == "scalar":
            nc.scalar.copy(dst[:, ts(tg, 512)], ps[:])
        else:
            nc.vector.tensor_copy(dst[:, ts(tg, 512)], ps[:])

    def emit_vaug_part(st, tg):
        vaug = st["vaug"]
        tps = [
            mm_psum.tile([P, 4, 64], BF16, tag="mm", name=f"vtr{h}")
            for h in range(HPC)
        ]
        for kk in range(4):
            kb = 4 * tg + kk
            for h in range(HPC):
                nc.tensor.transpose(
                    tps[h][:, kk, :],
                    st["vt"][64 * h : 64 * h + 64, ts(kb, KB)],
                    ident[64 * h : 64 * h + 64, :],
                )
        for h in range(HPC):
            nc.vector.tensor_copy(
                vaug[h][:, 4 * tg : 4 * tg + 4, 0:64], tps[h][:]
            )

    def emit_proj_chunk(st, g, tc4, copy_eng):
        b, ohat = st["b"], st["ohat"]
        t0 = G * g + P * tc4
        o_sb = out_pool.tile([P, C], BF16, tag="osb", name=f"osb{b}{g}{tc4}")
        for n in range(C // 512):
            pj = mm_psum.tile([P, 512], F32, tag="mm", name=f"pj{n}")
            nc.tensor.matmul(
                pj[:],
                ohat[:, t0 : t0 + P],
                wpt_sb[:, ts(n, 512)],
                start=True,
                stop=True,
            )
            if copy_eng == "scalar":
                nc.scalar.copy(o_sb[:, ts(n, 512)], pj[:])
            else:
                nc.vector.tensor_copy(o_sb[:, ts(n, 512)], pj[:])
        nc.sync.dma_start(out[b, t0 : t0 + P, :], o_sb[:])

    # ---------- filler unit queue ----------
    # each unit: (key, fn); key=(b, tg) for qkv units (forced before the
    # attention group that needs them), (9, 9) for proj units (never forced)
    units = deque()

    def pop_units(maxn):
        n = 0
        while units and n < maxn:
            _, fn = units.popleft()
            fn()
            n += 1

    def force_units(b, g):
        while units and units[0][0] <= (b, g):
            _, fn = units.popleft()
            fn()

    def queue_qkv(st, tg):
        for which in ("q", "k", "v"):
            units.append(
                ((st["b"], tg),
                 lambda st=st, w=which, tg=tg: emit_qkv_group(st, w, tg, "vector"))
            )
        units.append(((st["b"], tg), lambda st=st, tg=tg: emit_vaug_part(st, tg)))

    def queue_proj(st, g, copy_eng="vector"):
        for tc4 in range(G // P):
            units.append(
                ((9, 9),
                 lambda st=st, g=g, tc4=tc4, e=copy_eng: emit_proj_chunk(st, g, tc4, e))
            )

    # ---------- attention for one (b, g) with one-jg S/exp -> O skew ----------
    def emit_attn_g(st, g):
        b, qt, kt, vaug, ohat = st["b"], st["qt"], st["kt"], st["vaug"], st["ohat"]
        n_j = 4 * g + 4
        n_jg = n_j // 2
        otps_h = [
            ot_psum.tile([P, G], F32, tag="ot", name=f"ot{b}{g}{h}")
            for h in range(HPC)
        ]
        pend = None  # (js, pt_h) waiting for O^T

        def emit_s_exp(jg):
            js = (2 * jg, 2 * jg + 1)
            diag = 2 * jg >= 4 * g
            stps_h = [
                st_psum.tile([P, 2, G], F32, tag="st", name=f"st{b}{g}{h}")
                for h in range(HPC)
            ]
            pt_h = [
                pt_pool.tile([P, 2, G], BF16, tag=f"pt{h}", name=f"pt{b}{g}{h}")
                for h in range(HPC)
            ]
            for idx, j in enumerate(js):
                r = j - 4 * g
                q0 = 128 * r if r >= 0 else 0
                for h in range(HPC):
                    hb = 64 * h
                    nc.tensor.matmul(
                        stps_h[h][:, idx, q0:G],
                        kt[hb : hb + 64, ts(j, KB)],
                        qt[hb : hb + 64, G * g + q0 : G * (g + 1)],
                        start=True,
                        stop=True,
                    )
            for h in range(HPC):
                # full-width exp: columns left of a diagonal block's q0 hold
                # stale psum values; their exp lands in pt but is never read
                # by the O^T matmuls (restricted to q0:G)
                nc.scalar.activation(
                    pt_h[h][:, :, :],
                    stps_h[h][:, :, :],
                    mybir.ActivationFunctionType.Exp,
                    scale=SCALE,
                )
            if diag:
                for idx, j in enumerate(js):
                    q0 = 128 * (j - 4 * g)
                    for h in range(HPC):
                        nc.gpsimd.tensor_tensor(
                            pt_h[h][:, idx, q0 : q0 + 128],
                            pt_h[h][:, idx, q0 : q0 + 128],
                            tri_sb[:],
                            mybir.AluOpType.mult,
                        )
            return (js, pt_h)

        def emit_o(pend):
            js, pt_h = pend
            for idx, j in enumerate(js):
                r = j - 4 * g
                q0 = 128 * r if r >= 0 else 0
                for h in range(HPC):
                    nc.tensor.matmul(
                        otps_h[h][:, q0:G],
                        vaug[h][:, j, :],
                        pt_h[h][:, idx, q0:G],
                        start=(j == 0),
                        stop=(j == n_j - 1),
                    )

        for jg in range(n_jg + 1):
            if jg < n_jg:
                pend_new = emit_s_exp(jg)
            pop_units(2)
            if pend is not None:
                emit_o(pend)
            pend = pend_new if jg < n_jg else None

        # normalize: O rows (0:64 per head) / l rows (64:128 per head)
        l_sb = norm_pool.tile([P, G], F32, tag="lsb", name=f"l{b}{g}")
        rinv = norm_pool.tile([P, G], F32, tag="rinv", name=f"r{b}{g}")
        stag = norm_pool.tile([P, G], F32, tag="stag", name=f"sg{b}{g}")
        for h in range(HPC):
            hb = 64 * h
            nc.vector.tensor_copy(stag[hb : hb + 64, :], otps_h[h][0:64, :])
            nc.vector.tensor_copy(l_sb[hb : hb + 64, :], otps_h[h][64:128, :])
        nc.vector.reciprocal_approx_fast(rinv[:], l_sb[:])
        nc.vector.tensor_tensor(
            ohat[:, ts(g, G)], stag[:], rinv[:], mybir.AluOpType.mult
        )

    # ================= emission =================
    st = {0: new_state(0), 1: new_state(1)}

    # eager: qkv b0 tg0 (copies split scalar/vector: both engines idle here)
    for i, which in enumerate(("q", "k", "v")):
        emit_qkv_group(st[0], which, 0, "scalar" if i % 2 == 0 else "vector")
    emit_vaug_part(st[0], 0)

    for b in (0,):
        for tg in (1, 2, 3):
            queue_qkv(st[b], tg)
    for tg in range(NG):
        queue_qkv(st[1], tg)

    for b in (0, 1):
        for g in range(NG):
            force_units(b, g)
            emit_attn_g(st[b], g)
            queue_proj(st[b], g)

    # tail drain: alternate copy engines (no exps left, scalar is free)
    ntail = 0
    while units:
        _, fn = units.popleft()
        fn()
        ntail += 1
    ctx.close()


def _build():
    if "nc" in _nc_cache:
        return _nc_cache["nc"]
    nc = bacc.Bacc("TRN2", target_bir_lowering=False, debug=False)
    with tile.TileContext(nc) as tc:
        _emit(tc)
    nc.compile()
    _nc_cache["nc"] = nc
    return nc


def _make_in_maps(x, wq, wk, wv, w_proj):
    xt = np.ascontiguousarray(x.transpose(0, 2, 1)).astype(np.float32)
    import ml_dtypes

    tri = np.triu(np.ones((P, P), dtype=np.float32)).astype(ml_dtypes.bfloat16)
    ident = np.tile(np.eye(64, dtype=np.float32), (2, 1)).astype(ml_dtypes.bfloat16)
    ones = np.ones((P, NKB, 64), dtype=np.float32).astype(ml_dtypes.bfloat16)
    in_maps = []
    for c in range(NCORES):
        h0 = HPC * c
        in_maps.append(
            {
                "xt": xt,
                "wq2": np.ascontiguousarray(
                    np.concatenate([wq[h0 + i] for i in range(HPC)], axis=1)
                ).astype(np.float32),
                "wk2": np.ascontiguousarray(
                    np.concatenate([wk[h0 + i] for i in range(HPC)], axis=1)
                ).astype(np.float32),
                "wv2": np.ascontiguousarray(
                    np.concatenate([wv[h0 + i] for i in range(HPC)], axis=1)
                ).astype(np.float32),
                "wpt": np.ascontiguousarray(
                    w_proj[:, 128 * c : 128 * (c + 1)].T
                ).astype(np.float32),
                "tri": tri,
                "ident": ident,
                "ones": ones,
            }
        )
    return in_maps


def kernel(x, wq, wk, wv, w_proj, b_proj):
    x = np.asarray(x, dtype=np.float32)
    wq = np.asarray(wq, dtype=np.float32)
    wk = np.asarray(wk, dtype=np.float32)
    wv = np.asarray(wv, dtype=np.float32)
    w_proj = np.asarray(w_proj, dtype=np.float32)
    b_proj = np.asarray(b_proj, dtype=np.float32)

    nc = _build()
    in_maps = _make_in_maps(x, wq, wk, wv, w_proj)
    res = run_bass_kernel_spmd(nc, in_maps, core_ids=list(range(NCORES)))
    acc = np.zeros((B, T, C), dtype=np.float64)
    for r in res.results:
        acc += np.asarray(r["out"], dtype=np.float64)
    return (acc + b_proj).astype(np.float32)


# revision 7
# speedup vs baseline: 1.4335x; 1.4335x over previous
"""Multi-head causal attention (B=2, T=2048, C=1024, H=16, HS=64) on 8 TRN2
NeuronCores.

Sharding: 2 heads per core (tensor parallel). Each core receives the full
(pre-transposed) activations xT [B, C, T], its 2 heads' QKV weight slices
packed [C, 128], and its 128-column slice of w_proj transposed [128, C].
Each core computes a partial output [B, T, C] in bf16; the host sums the 8
partials and adds b_proj.

Per-core kernel (matmuls in float32r):
  - QT/KT [128(2 heads x 64), T] via lhsT=weight chunks, rhs=xT chunks.
  - V_aug [keys, 2, j, 64]: V (vo=0, via PE-transpose of VT) | ones (vo=1).
  - Flash-style causal attention in transposed layout: S^T[keys, q] blocks
    via lhsT=KT block, rhs=QT slice; exp on ScalarE (no max subtraction --
    scores are O(1) by construction); O^T = [V|1].T @ P^T accumulated over
    key blocks gives both O rows (0:64) and the softmax sums l (64:128).
  - Triangular masking of diagonal blocks via bf16 multiply on GpSimd.
  - Normalize with reciprocal_approx_fast; proj via lhsT=OhatT chunks.

Scheduling: the PE p-state ramps to 2.4 GHz only after ~3us of gap-free
execution and resets on any idle, so the emission interleaves one "filler"
PE unit (a QKV 512-col chain or a proj chunk) into every attention jg slot,
and skews O^T one jg behind S^T/exp so the PE never waits on ScalarE.
"""

import math
import sys
from collections import deque
from contextlib import ExitStack

if "/opt/trn_rl_repo" not in sys.path:
    sys.path.insert(0, "/opt/trn_rl_repo")

import numpy as np

import concourse.mybir as mybir
import concourse.tile as tile
from concourse import bacc
from concourse.bass import ts
from concourse.bass_utils import run_bass_kernel_spmd
from concourse.tile_rust import add_dep_helper

B, T, C = 2, 2048, 1024
H, HS = 16, 64
NCORES = 8
HPC = H // NCORES  # heads per core
P = 128
G = 512  # q-group size
NG = T // G
KB = 128  # key block
NKB = T // KB
NPO = C // P  # contraction chunks
F32 = mybir.dt.float32
F32R = mybir.dt.float32r
BF16 = mybir.dt.bfloat16
SCALE = float(HS) ** -0.5

_nc_cache = {}


def _emit(tc):
    nc = tc.nc
    xt = nc.dram_tensor("xt", [B, C, T], BF16, kind="ExternalInput").ap()
    wq2 = nc.dram_tensor("wq2", [C, 128], BF16, kind="ExternalInput").ap()
    wk2 = nc.dram_tensor("wk2", [C, 128], BF16, kind="ExternalInput").ap()
    wv2 = nc.dram_tensor("wv2", [C, 128], BF16, kind="ExternalInput").ap()
    wpt = nc.dram_tensor("wpt", [128, C], BF16, kind="ExternalInput").ap()
    tri = nc.dram_tensor("tri", [P, P], BF16, kind="ExternalInput").ap()
    identd = nc.dram_tensor("ident", [P, 64], BF16, kind="ExternalInput").ap()
    onesd = nc.dram_tensor("ones", [P, NKB, 64], BF16, kind="ExternalInput").ap()
    out = nc.dram_tensor("out", [B, T, C], BF16, kind="ExternalOutput").ap()

    ctx = ExitStack()
    persist = ctx.enter_context(tc.tile_pool(name="persist", bufs=1))
    xt_pool = ctx.enter_context(tc.tile_pool(name="xtp", bufs=3))
    qk_pool = ctx.enter_context(tc.tile_pool(name="qkp", bufs=2))
    vt_pool = ctx.enter_context(tc.tile_pool(name="vtp", bufs=2))
    vaug_pool = ctx.enter_context(tc.tile_pool(name="vaugp", bufs=2))
    pt_pool = ctx.enter_context(tc.tile_pool(name="ptp", bufs=4))
    norm_pool = ctx.enter_context(tc.tile_pool(name="normp", bufs=2))
    ohat_pool = ctx.enter_context(tc.tile_pool(name="ohatp", bufs=2))
    out_pool = ctx.enter_context(tc.tile_pool(name="outp", bufs=3))
    st_psum = ctx.enter_context(tc.tile_pool(name="stps", bufs=2, space="PSUM"))
    ot_psum = ctx.enter_context(tc.tile_pool(name="otps", bufs=2, space="PSUM"))
    mm_psum = ctx.enter_context(tc.tile_pool(name="mmps", bufs=2, space="PSUM"))

    wq_sb = persist.tile([P, NPO, 128], BF16, tag="wq")
    wk_sb = persist.tile([P, NPO, 128], BF16, tag="wk")
    wv_sb = persist.tile([P, NPO, 128], BF16, tag="wv")
    wpt_sb = persist.tile([P, C], BF16, tag="wpt")
    tri_sb = persist.tile([P, P], BF16, tag="tri")
    ident = persist.tile([P, 64], BF16, tag="ident")

    nc.sync.dma_start(wq_sb[:], wq2.rearrange("(po pi) d -> pi po d", pi=P))
    nc.sync.dma_start(wk_sb[:], wk2.rearrange("(po pi) d -> pi po d", pi=P))
    nc.sync.dma_start(wv_sb[:], wv2.rearrange("(po pi) d -> pi po d", pi=P))
    nc.sync.dma_start(tri_sb[:], tri[:])
    nc.sync.dma_start(ident[:], identd[:])
    nc.sync.dma_start(wpt_sb[:], wpt[:])

    # ---- xt loading: per-tg tiles, loaded in two half-chunks (4 po each) ----
    xt_tiles = {}
    xt_dmas = []

    def load_xt_tg(b, tg):
        t = xt_pool.tile([P, NPO, 512], BF16, tag="xt", name=f"xt{b}{tg}")
        xt_tiles[(b, tg)] = t
        src = xt[b, :, ts(tg, 512)].rearrange("(po pi) t -> pi po t", pi=P)
        for half in range(2):
            po0 = 4 * half
            i = nc.sync.dma_start(t[:, po0 : po0 + 4, :], src[:, po0 : po0 + 4, :])
            if len(xt_dmas) >= 3:
                add_dep_helper(i.ins, xt_dmas[-3].ins, sync=True)
            xt_dmas.append(i)

    # just-in-time order: b0 tg0-2, then b1 tg0 before b0 tg3 so fillers
    # popped early in b0's attention never stall the PE queue
    for b, tg in [(0, 0), (0, 1), (0, 2), (1, 0), (0, 3), (1, 1), (1, 2), (1, 3)]:
        load_xt_tg(b, tg)

    def new_state(b):
        st = {
            "b": b,
            "qt": qk_pool.tile([P, T], BF16, tag="qt", name=f"qt{b}"),
            "kt": qk_pool.tile([P, T], BF16, tag="kt", name=f"kt{b}"),
            "vt": vt_pool.tile([P, T], BF16, tag="vt", name=f"vt{b}"),
            "ohat": ohat_pool.tile([P, T], BF16, tag="ohat", name=f"oh{b}"),
            "vaug": [],
        }
        for h in range(HPC):
            va = vaug_pool.tile(
                [P, NKB, 128], BF16, tag=f"vaug{h}", name=f"va{b}{h}"
            )
            i = nc.sync.dma_start(va[:, :, 64:128], onesd[:])
            add_dep_helper(i.ins, xt_dmas[0].ins, sync=True)
            st["vaug"].append(va)
        return st

    # ---------- building blocks ----------
    def emit_qkv_group(st, which, tg, copy_eng):
        w_sb, dst = {
            "q": (wq_sb, st["qt"]),
            "k": (wk_sb, st["kt"]),
            "v": (wv_sb, st["vt"]),
        }[which]
        ps = mm_psum.tile([P, 512], F32, tag="mm", name=f"qkv{which}{tg}")
        xtt = xt_tiles[(st["b"], tg)]
        for po in range(NPO):
            nc.tensor.matmul(
                ps[:],
                w_sb[:, po, :],
                xtt[:, po, :],
                start=(po == 0),
                stop=(po == NPO - 1),
            )
        if copy_eng == "scalar":
            nc.scalar.copy(dst[:, ts(tg, 512)], ps[:])
        else:
            nc.vector.tensor_copy(dst[:, ts(tg, 512)], ps[:])

    def emit_vaug_part(st, tg):
        vaug = st["vaug"]
        tps = [
            mm_psum.tile([P, 4, 64], BF16, tag="mm", name=f"vtr{h}")
            for h in range(HPC)
        ]
        for kk in range(4):
            kb = 4 * tg + kk
            for h in range(HPC):
                nc.tensor.transpose(
                    tps[h][:, kk, :],
                    st["vt"][64 * h : 64 * h + 64, ts(kb, KB)],
                    ident[64 * h : 64 * h + 64, :],
                )
        for h in range(HPC):
            nc.vector.tensor_copy(
                vaug[h][:, 4 * tg : 4 * tg + 4, 0:64], tps[h][:]
            )

    def emit_proj_chunk(st, g, tc4, copy_eng):
        b, ohat = st["b"], st["ohat"]
        t0 = G * g + P * tc4
        o_sb = out_pool.tile([P, C], BF16, tag="osb", name=f"osb{b}{g}{tc4}")
        for n in range(C // 512):
            pj = mm_psum.tile([P, 512], F32, tag="mm", name=f"pj{n}")
            nc.tensor.matmul(
                pj[:],
                ohat[:, t0 : t0 + P],
                wpt_sb[:, ts(n, 512)],
                start=True,
                stop=True,
            )
            if copy_eng == "scalar":
                nc.scalar.copy(o_sb[:, ts(n, 512)], pj[:])
            else:
                nc.vector.tensor_copy(o_sb[:, ts(n, 512)], pj[:])
        nc.sync.dma_start(out[b, t0 : t0 + P, :], o_sb[:])

    # ---------- filler unit queue ----------
    # each unit: (key, fn); key=(b, tg) for qkv units (forced before the
    # attention group that needs them), (9, 9) for proj units (never forced)
    units = deque()

    def pop_units(maxn):
        n = 0
        while units and n < maxn:
            _, fn = units.popleft()
            fn()
            n += 1

    def force_units(b, g):
        while units and units[0][0] <= (b, g):
            _, fn = units.popleft()
            fn()

    def queue_qkv(st, tg):
        for which in ("q", "k", "v"):
            units.append(
                ((st["b"], tg),
                 lambda st=st, w=which, tg=tg: emit_qkv_group(st, w, tg, "vector"))
            )
        units.append(((st["b"], tg), lambda st=st, tg=tg: emit_vaug_part(st, tg)))

    def queue_proj(st, g, copy_eng="vector"):
        for tc4 in range(G // P):
            units.append(
                ((9, 9),
                 lambda st=st, g=g, tc4=tc4, e=copy_eng: emit_proj_chunk(st, g, tc4, e))
            )

    # ---------- attention for one (b, g) with one-jg S/exp -> O skew ----------
    def emit_attn_g(st, g):
        b, qt, kt, vaug, ohat = st["b"], st["qt"], st["kt"], st["vaug"], st["ohat"]
        n_j = 4 * g + 4
        n_jg = n_j // 2
        otps_h = [
            ot_psum.tile([P, G], F32, tag="ot", name=f"ot{b}{g}{h}")
            for h in range(HPC)
        ]
        pend = None  # (js, pt_h) waiting for O^T

        def emit_s_exp(jg):
            js = (2 * jg, 2 * jg + 1)
            diag = 2 * jg >= 4 * g
            stps_h = [
                st_psum.tile([P, 2, G], F32, tag="st", name=f"st{b}{g}{h}")
                for h in range(HPC)
            ]
            pt_h = [
                pt_pool.tile([P, 2, G], BF16, tag=f"pt{h}", name=f"pt{b}{g}{h}")
                for h in range(HPC)
            ]
            for idx, j in enumerate(js):
                r = j - 4 * g
                q0 = 128 * r if r >= 0 else 0
                for h in range(HPC):
                    hb = 64 * h
                    nc.tensor.matmul(
                        stps_h[h][:, idx, q0:G],
                        kt[hb : hb + 64, ts(j, KB)],
                        qt[hb : hb + 64, G * g + q0 : G * (g + 1)],
                        start=True,
                        stop=True,
                    )
            for h in range(HPC):
                # full-width exp: columns left of a diagonal block's q0 hold
                # stale psum values; their exp lands in pt but is never read
                # by the O^T matmuls (restricted to q0:G)
                nc.scalar.activation(
                    pt_h[h][:, :, :],
                    stps_h[h][:, :, :],
                    mybir.ActivationFunctionType.Exp,
                    scale=SCALE,
                )
            if diag:
                for idx, j in enumerate(js):
                    q0 = 128 * (j - 4 * g)
                    for h in range(HPC):
                        nc.gpsimd.tensor_tensor(
                            pt_h[h][:, idx, q0 : q0 + 128],
                            pt_h[h][:, idx, q0 : q0 + 128],
                            tri_sb[:],
                            mybir.AluOpType.mult,
                        )
            return (js, pt_h)

        def emit_o(pend):
            js, pt_h = pend
            for idx, j in enumerate(js):
                r = j - 4 * g
                q0 = 128 * r if r >= 0 else 0
                for h in range(HPC):
                    nc.tensor.matmul(
                        otps_h[h][:, q0:G],
                        vaug[h][:, j, :],
                        pt_h[h][:, idx, q0:G],
                        start=(j == 0),
                        stop=(j == n_j - 1),
                    )

        for jg in range(n_jg + 1):
            if jg < n_jg:
                pend_new = emit_s_exp(jg)
            pop_units(2)
            if pend is not None:
                emit_o(pend)
            pend = pend_new if jg < n_jg else None

        # normalize: O rows (0:64 per head) / l rows (64:128 per head)
        l_sb = norm_pool.tile([P, G], F32, tag="lsb", name=f"l{b}{g}")
        rinv = norm_pool.tile([P, G], F32, tag="rinv", name=f"r{b}{g}")
        stag = norm_pool.tile([P, G], F32, tag="stag", name=f"sg{b}{g}")
        for h in range(HPC):
            hb = 64 * h
            nc.vector.tensor_copy(stag[hb : hb + 64, :], otps_h[h][0:64, :])
            nc.vector.tensor_copy(l_sb[hb : hb + 64, :], otps_h[h][64:128, :])
        nc.vector.reciprocal_approx_fast(rinv[:], l_sb[:])
        nc.vector.tensor_tensor(
            ohat[:, ts(g, G)], stag[:], rinv[:], mybir.AluOpType.mult
        )

    # ================= emission =================
    st = {0: new_state(0), 1: new_state(1)}

    # eager: qkv b0 tg0 (copies split scalar/vector: both engines idle here)
    for i, which in enumerate(("q", "k", "v")):
        emit_qkv_group(st[0], which, 0, "scalar" if i % 2 == 0 else "vector")
    emit_vaug_part(st[0], 0)

    for b in (0,):
        for tg in (1, 2, 3):
            queue_qkv(st[b], tg)
    for tg in range(NG):
        queue_qkv(st[1], tg)

    for b in (0, 1):
        for g in range(NG):
            force_units(b, g)
            emit_attn_g(st[b], g)
            queue_proj(st[b], g)

    # tail drain: alternate copy engines (no exps left, scalar is free)
    ntail = 0
    while units:
        _, fn = units.popleft()
        fn()
        ntail += 1
    ctx.close()


def _build():
    if "nc" in _nc_cache:
        return _nc_cache["nc"]
    nc = bacc.Bacc("TRN2", target_bir_lowering=False, debug=False)
    with tile.TileContext(nc) as tc:
        _emit(tc)
    nc.compile()
    _nc_cache["nc"] = nc
    return nc


def _make_in_maps(x, wq, wk, wv, w_proj):
    import ml_dtypes

    bf16 = ml_dtypes.bfloat16
    xt = np.ascontiguousarray(x.transpose(0, 2, 1)).astype(bf16)
    tri = np.triu(np.ones((P, P), dtype=np.float32)).astype(bf16)
    ident = np.tile(np.eye(64, dtype=np.float32), (2, 1)).astype(bf16)
    ones = np.ones((P, NKB, 64), dtype=np.float32).astype(bf16)
    in_maps = []
    for c in range(NCORES):
        h0 = HPC * c
        in_maps.append(
            {
                "xt": xt,
                "wq2": np.ascontiguousarray(
                    np.concatenate([wq[h0 + i] for i in range(HPC)], axis=1)
                ).astype(bf16),
                "wk2": np.ascontiguousarray(
                    np.concatenate([wk[h0 + i] for i in range(HPC)], axis=1)
                ).astype(bf16),
                "wv2": np.ascontiguousarray(
                    np.concatenate([wv[h0 + i] for i in range(HPC)], axis=1)
                ).astype(bf16),
                "wpt": np.ascontiguousarray(
                    w_proj[:, 128 * c : 128 * (c + 1)].T
                ).astype(bf16),
                "tri": tri,
                "ident": ident,
                "ones": ones,
            }
        )
    return in_maps


def kernel(x, wq, wk, wv, w_proj, b_proj):
    x = np.asarray(x, dtype=np.float32)
    wq = np.asarray(wq, dtype=np.float32)
    wk = np.asarray(wk, dtype=np.float32)
    wv = np.asarray(wv, dtype=np.float32)
    w_proj = np.asarray(w_proj, dtype=np.float32)
    b_proj = np.asarray(b_proj, dtype=np.float32)

    nc = _build()
    in_maps = _make_in_maps(x, wq, wk, wv, w_proj)
    res = run_bass_kernel_spmd(nc, in_maps, core_ids=list(range(NCORES)))
    acc = np.zeros((B, T, C), dtype=np.float64)
    for r in res.results:
        acc += np.asarray(r["out"], dtype=np.float64)
    return (acc + b_proj).astype(np.float32)
